# revision 18
# baseline (speedup 1.0000x reference)
"""Trainium2 Bass kernel for a single-step attention-LSTM decoder.

Distribution over 8 NeuronCores (one program, per-core data):
  - attention: batch-sharded (8 batches/core); context AllGather
  - LSTM gates: H-sharded (128 hidden units/core, pre-transposed weight
    column slices); h1 AllGather
  - output projection + log-softmax: vocab-sharded (4096 padded rows/core,
    pre-transposed W_out shard streamed as the moving operand);
    AllReduce of the softmax normalizer

kernel(**inputs) takes the full unsharded inputs and returns the full
outputs, sharding/gathering on the host around one SPMD device launch.
"""

import sys

sys.path.insert(0, "/opt/trn_rl_repo")

import numpy as np

# ---- problem constants (hardcoded per contract) ----
V, H, E = 32000, 1024, 1024
B, S = 64, 128
NC = 8
VPAD = 32768            # vocab padded to NC * 4096
VS = VPAD // NC         # vocab shard per core = 4096
BS = B // NC            # batch shard per core = 8
HS = H // NC            # hidden shard per core = 128
GS = 4 * HS             # gate columns per core = 512
P = 128

# matmul dtype knobs: float32r streams 4x faster when the moving free dim
# is >= 256; precision is validated against the fp32 reference in test.py.
WOUT_F32R = False
GATES_F32R = False

_cached = {}


def _dt():
    import concourse.mybir as mybir
    return mybir


def _build_program():
    """Build + compile the SPMD Bass program (same program on all cores)."""
    from concourse import bacc
    import concourse.bass as bass
    import concourse.mybir as mybir
    import concourse.tile as tile

    f32 = mybir.dt.float32
    f32r = mybir.dt.float32r
    i32 = mybir.dt.int32
    AF = mybir.ActivationFunctionType
    OP = mybir.AluOpType

    nc = bacc.Bacc("TRN2", target_bir_lowering=False, debug=False, num_devices=NC)

    # ---- external inputs (per-core contents differ where sharded) ----
    enc_d = nc.dram_tensor("enc", [BS, S, H], f32, kind="ExternalInput")
    gdt = f32r if GATES_F32R else f32
    wdt = f32r if WOUT_F32R else f32
    wih_d = nc.dram_tensor("wihT", [E + H, GS], gdt, kind="ExternalInput")
    whh_d = nc.dram_tensor("whhT", [H, GS], gdt, kind="ExternalInput")
    bih_d = nc.dram_tensor("bihr", [1, GS], f32, kind="ExternalInput")
    bhh_d = nc.dram_tensor("bhhr", [1, GS], f32, kind="ExternalInput")
    wo_d = nc.dram_tensor("woT", [H, VS], wdt, kind="ExternalInput")
    bo_d = nc.dram_tensor("boS", [1, VS], f32, kind="ExternalInput")
    emb_d = nc.dram_tensor("emb", [V, E], f32, kind="ExternalInput")
    idx_d = nc.dram_tensor("xidx", [B, 1], i32, kind="ExternalInput")
    h0T_d = nc.dram_tensor("h0T", [H, B], gdt, kind="ExternalInput")
    c0_d = nc.dram_tensor("c0s", [B, HS], f32, kind="ExternalInput")
    len_d = nc.dram_tensor("lens", [B, 1], i32, kind="ExternalInput")
    we_d = nc.dram_tensor("weR", [1, H], f32, kind="ExternalInput")
    id_d = nc.dram_tensor("ident", [P, P], f32, kind="ExternalInput")
    on_d = nc.dram_tensor("onesr", [1, B], f32, kind="ExternalInput")

    # ---- external outputs ----
    lp_d = nc.dram_tensor("lp", [B, VS], f32, kind="ExternalOutput")
    h1_d = nc.dram_tensor("h1s", [B, HS], f32, kind="ExternalOutput")
    c1_d = nc.dram_tensor("c1s", [B, HS], f32, kind="ExternalOutput")
    aw_d = nc.dram_tensor("aws", [BS, S], f32, kind="ExternalOutput")

    HC = H // P            # 8 h-chunks
    FC = (E + H) // P      # 16 feature chunks for W_ih
    NVT = VS // 512        # 8 vocab tiles of 512

    with tile.TileContext(nc) as tc:
        with tc.tile_pool(name="const", bufs=1) as cpool, \
             tc.tile_pool(name="enc", bufs=BS) as encp, \
             tc.tile_pool(name="wts", bufs=1) as wtp, \
             tc.tile_pool(name="wo", bufs=2) as wop, \
             tc.tile_pool(name="work", bufs=1) as wk, \
             tc.tile_pool(name="scr", bufs=2) as scr, \
             tc.tile_pool(name="dram", bufs=1, space="DRAM") as dp:

            # ---------- constants / small inputs ----------
            ident = cpool.tile([P, P], f32)
            nc.sync.dma_start(ident[:], id_d[:])
            onesr = cpool.tile([1, B], f32)
            nc.sync.dma_start(onesr[:], on_d[:])
            web = cpool.tile([P, H], f32)
            nc.sync.dma_start(web[:], we_d[:].to_broadcast([P, H]))
            h0T = cpool.tile([P, HC, B], gdt)
            nc.sync.dma_start(h0T[:], h0T_d[:].rearrange("(c p) b -> p c b", p=P))
            c0s = cpool.tile([B, HS], f32)
            nc.sync.dma_start(c0s[:], c0_d[:])
            lens_t = cpool.tile([B, 1], i32)
            nc.sync.dma_start(lens_t[:], len_d[:])
            bihr = cpool.tile([1, GS], f32)
            nc.sync.dma_start(bihr[:], bih_d[:])
            bhhr = cpool.tile([1, GS], f32)
            nc.sync.dma_start(bhhr[:], bhh_d[:])
            bos = cpool.tile([1, VS], f32)
            nc.sync.dma_start(bos[:], bo_d[:])
            idx_t = cpool.tile([B, 1], i32)
            nc.sync.dma_start(idx_t[:], idx_d[:])

            # embedding gather (rows of emb by token id)
            xemb = wk.tile([B, E], f32)
            nc.gpsimd.indirect_dma_start(
                out=xemb[:], out_offset=None, in_=emb_d[:],
                in_offset=bass.IndirectOffsetOnAxis(ap=idx_t[:, :1], axis=0),
            )

            # ---------- encoder shard ----------
            enc_t = []
            for b in range(BS):
                e = encp.tile([S, H], f32, tag="enc")
                nc.sync.dma_start(e[:], enc_d[b])
                enc_t.append(e)

            # ---------- gate weights ----------
            wih_t = wtp.tile([P, FC, GS], gdt, tag="wih")
            nc.sync.dma_start(wih_t[:], wih_d[:].rearrange("(c p) g -> p c g", p=P))
            whh_t = wtp.tile([P, HC, GS], gdt, tag="whh")
            nc.sync.dma_start(whh_t[:], whh_d[:].rearrange("(c p) g -> p c g", p=P))

            # ---------- W_out^T strips (streamed; rotating pool) ----------
            wo_strips = []
            for hc in range(HC):
                st = wop.tile([P, VS], wdt, tag="wostrip")
                nc.sync.dma_start(st[:], wo_d[hc * P:(hc + 1) * P, :])
                wo_strips.append(st)

            # dram bounce buffers for collectives
            ctx_in = dp.tile([BS, H], f32, tag="ctx_in")
            ctx_out = dp.tile([B, H], f32, tag="ctx_out")
            h1_in = dp.tile([B, HS], f32, tag="h1_in")
            h1_out = dp.tile([NC * B, HS], f32, tag="h1_out")
            s_in = dp.tile([B, 1], f32, tag="s_in")
            s_out = dp.tile([B, 1], f32, tag="s_out")

            # ================= attention =================
            with tc.tile_pool(name="psA", bufs=2, space="PSUM") as psA:
                # e-scores: fused mult+reduce on DVE -> score columns [s, b]
                sc_cols = wk.tile([S, BS], f32)
                for b in range(BS):
                    ttr_o = wk.tile([S, H], f32, tag="ttr")
                    nc.vector.scalar_tensor_tensor(
                        out=ttr_o[:], in0=enc_t[b][:], scalar=1.0, in1=web[:],
                        op0=OP.mult, op1=OP.mult,
                        accum_out=sc_cols[:, b:b + 1],
                    )
                # transpose score columns -> rows [b, s]
                ps = psA.tile([BS, S], f32, tag="pT")
                nc.tensor.transpose(ps[:], sc_cols[:], ident[:])
                scores = wk.tile([BS, S], f32)
                nc.scalar.copy(scores[:], ps[:])

                # row softmax (shift-invariant: h0/bias terms omitted)
                m8 = wk.tile([BS, 1], f32)
                nc.vector.reduce_max(m8[:], scores[:], axis=mybir.AxisListType.X)
                nm8 = wk.tile([BS, 1], f32)
                nc.vector.tensor_scalar_mul(nm8[:], m8[:], -1.0)
                expt = wk.tile([BS, S], f32)
                s8 = wk.tile([BS, 1], f32)
                nc.scalar.activation(expt[:], scores[:], AF.Exp,
                                     bias=nm8[:, :1], scale=1.0,
                                     accum_out=s8[:, :1])
                r8 = wk.tile([BS, 1], f32)
                nc.vector.reciprocal(r8[:], s8[:])
                aw = wk.tile([BS, S], f32)
                nc.vector.tensor_scalar_mul(aw[:], expt[:], r8[:, :1])
                nc.sync.dma_start(aw_d[:], aw[:])

                # aw^T columns for the context matmuls
                psT = psA.tile([S, BS], f32, tag="pT")
                nc.tensor.transpose(psT[:], aw[:], ident[:BS, :BS])
                awT = wk.tile([S, BS], f32)
                nc.scalar.copy(awT[:], psT[:])

                # context (columns): ctxT_b[:, hc] = enc_b[:, hc]^T @ awT[:, b]
                for b in range(BS):
                    pc = psA.tile([P, HC], f32, tag="pC")
                    for hc in range(HC):
                        nc.tensor.matmul(
                            pc[:, hc:hc + 1],
                            lhsT=enc_t[b][:, hc * P:(hc + 1) * P],
                            rhs=awT[:, b:b + 1],
                            start=True, stop=True,
                        )
                    ctxTb = scr.tile([P, HC], f32, tag="ctxTb")
                    nc.scalar.copy(ctxTb[:], pc[:])
                    # transpose to natural layout and write this core's rows
                    pn = psA.tile([HC, P], f32, tag="pT")
                    nc.tensor.transpose(pn[:], ctxTb[:], ident[:])
                    ctxnb = scr.tile([HC, P], f32, tag="ctxnb")
                    nc.scalar.copy(ctxnb[:], pn[:])
                    nc.sync.dma_start(
                        ctx_in[b:b + 1, :].rearrange("one (c p) -> (one c) p", p=P),
                        ctxnb[:],
                    )

                # gather full context [B, H]
                nc.gpsimd.collective_compute(
                    "AllGather", OP.bypass,
                    replica_groups=[list(range(NC))],
                    ins=[ctx_in.opt()], outs=[ctx_out.opt()],
                )
                ctxf = wk.tile([B, H], f32)
                nc.sync.dma_start(ctxf[:], ctx_out[:])

                # x^T chunks (feature-major) for the gate matmuls
                xT = []
                for c in range(FC):
                    src = xemb if c < HC else ctxf
                    cc = c if c < HC else c - HC
                    px = psA.tile([P, B], f32, tag="pX")
                    nc.tensor.transpose(
                        px[:], src[:, cc * P:(cc + 1) * P], ident[:B, :B])
                    xt = wk.tile([P, B], gdt, tag=f"xT{c}")
                    nc.scalar.copy(xt[:], px[:])
                    xT.append(xt)

            # ================= LSTM gate slice =================
            with tc.tile_pool(name="psG", bufs=1, space="PSUM") as psG:
                pg = psG.tile([B, GS], f32)
                for c in range(FC):
                    nc.tensor.matmul(pg[:], lhsT=xT[c][:], rhs=wih_t[:, c, :],
                                     start=(c == 0), stop=False)
                for c in range(HC):
                    nc.tensor.matmul(pg[:], lhsT=h0T[:, c, :],
                                     rhs=whh_t[:, c, :],
                                     start=False, stop=False)
                nc.tensor.matmul(pg[:], lhsT=onesr[:1, :B], rhs=bihr[:1, :],
                                 start=False, stop=False)
                nc.tensor.matmul(pg[:], lhsT=onesr[:1, :B], rhs=bhhr[:1, :],
                                 start=False, stop=True)

                i_s = wk.tile([B, HS], f32)
                f_s = wk.tile([B, HS], f32)
                g_t = wk.tile([B, HS], f32)
                o_s = wk.tile([B, HS], f32)
                nc.scalar.activation(i_s[:], pg[:, 0 * HS:1 * HS], AF.Sigmoid)
                nc.scalar.activation(f_s[:], pg[:, 1 * HS:2 * HS], AF.Sigmoid)
                nc.scalar.activation(g_t[:], pg[:, 2 * HS:3 * HS], AF.Tanh)
                nc.scalar.activation(o_s[:], pg[:, 3 * HS:4 * HS], AF.Sigmoid)

                # zero-length mask as a per-batch scale
                lensf = wk.tile([B, 1], f32)
                nc.vector.tensor_copy(lensf[:], lens_t[:])
                msk = wk.tile([B, 1], f32)
                nc.scalar.sign(msk[:], lensf[:])

                t1 = wk.tile([B, HS], f32)
                nc.vector.tensor_mul(t1[:], f_s[:], c0s[:])
                t2 = wk.tile([B, HS], f32)
                nc.vector.tensor_mul(t2[:], i_s[:], g_t[:])
                c1 = wk.tile([B, HS], f32)
                nc.vector.tensor_add(c1[:], t1[:], t2[:])
                c1m = wk.tile([B, HS], f32)
                nc.scalar.mul(c1m[:], c1[:], msk[:, :1])
                th = wk.tile([B, HS], f32)
                nc.scalar.activation(th[:], c1m[:], AF.Tanh)
                h1 = wk.tile([B, HS], f32)
                nc.vector.tensor_mul(h1[:], o_s[:], th[:])

                nc.sync.dma_start(c1_d[:], c1m[:])
                nc.sync.dma_start(h1_d[:], h1[:])
                nc.sync.dma_start(h1_in[:], h1[:])

            nc.gpsimd.collective_compute(
                "AllGather", OP.bypass,
                replica_groups=[list(range(NC))],
                ins=[h1_in.opt()], outs=[h1_out.opt()],
            )

            # h1^T chunks [h, b] (chunk r comes from rank r's slice)
            h1T = []
            with tc.tile_pool(name="psH", bufs=2, space="PSUM") as psH:
                for r in range(NC):
                    blk = scr.tile([B, HS], f32, tag="h1blk")
                    nc.sync.dma_start(blk[:], h1_out[r * B:(r + 1) * B, :])
                    ph = psH.tile([HS, B], f32, tag="pH")
                    nc.tensor.transpose(ph[:], blk[:], ident[:B, :B])
                    ht = wk.tile([HS, B], wdt, tag=f"h1T{r}")
                    nc.scalar.copy(ht[:], ph[:])
                    h1T.append(ht)

            # ================= vocab-sharded projection + log-softmax =====
            with tc.tile_pool(name="psW", bufs=1, space="PSUM") as psW:
                pls = [psW.tile([B, 512], f32, tag=f"pl{v}", name=f"pl{v}")
                       for v in range(NVT)]
                for hc in range(HC):
                    for vt in range(NVT):
                        nc.tensor.matmul(
                            pls[vt][:], lhsT=h1T[hc][:],
                            rhs=wo_strips[hc][:, vt * 512:(vt + 1) * 512],
                            start=(hc == 0), stop=False)
                for vt in range(NVT):
                    nc.tensor.matmul(pls[vt][:], lhsT=onesr[:1, :B],
                                     rhs=bos[:1, vt * 512:(vt + 1) * 512],
                                     start=False, stop=True)

                sep = wk.tile([B, NVT], f32)
                for vt in range(NVT):
                    ex = scr.tile([B, 512], f32, tag="expscr")
                    nc.scalar.activation(ex[:], pls[vt][:], AF.Exp,
                                         accum_out=sep[:, vt:vt + 1])
                sloc = wk.tile([B, 1], f32)
                nc.vector.reduce_sum(sloc[:], sep[:], axis=mybir.AxisListType.X)
                nc.sync.dma_start(s_in[:], sloc[:])
                nc.gpsimd.collective_compute(
                    "AllReduce", OP.add,
                    replica_groups=[list(range(NC))],
                    ins=[s_in.opt()], outs=[s_out.opt()],
                )
                stot = wk.tile([B, 1], f32)
                nc.sync.dma_start(stot[:], s_out[:])
                logS = wk.tile([B, 1], f32)
                nc.scalar.activation(logS[:], stot[:], AF.Ln)
                nlogS = wk.tile([B, 1], f32)
                nc.vector.tensor_scalar_mul(nlogS[:], logS[:], -1.0)
                for vt in range(NVT):
                    lp = scr.tile([B, 512], f32, tag="lpscr")
                    nc.scalar.activation(lp[:], pls[vt][:], AF.Identity,
                                         bias=nlogS[:, :1], scale=1.0)
                    nc.sync.dma_start(lp_d[:, vt * 512:(vt + 1) * 512], lp[:])

    nc.compile()
    return nc


def _get_program():
    if "nc" not in _cached:
        _cached["nc"] = _build_program()
    return _cached["nc"]


def _prep_inputs(input_batch, prev_h, prev_c, encoder_outputs, lengths,
                 emb, attn_w, attn_b, W_ih, W_hh, b_ih, b_hh, W_out, b_out):
    f = np.float32
    emb = np.ascontiguousarray(emb, dtype=f)
    enc = np.ascontiguousarray(encoder_outputs, dtype=f)
    h0 = np.asarray(prev_h, dtype=f)[0]                 # [B, H]
    c0 = np.asarray(prev_c, dtype=f)[0]                 # [B, H]
    h0T = np.ascontiguousarray(h0.T)                    # [H, B]
    idx = np.ascontiguousarray(np.asarray(input_batch).reshape(B, 1).astype(np.int32))
    lens = np.ascontiguousarray(np.asarray(lengths).reshape(B, 1).astype(np.int32))
    weR = np.ascontiguousarray(np.asarray(attn_w, dtype=f)[H:].reshape(1, H))

    wihT = np.ascontiguousarray(np.asarray(W_ih, dtype=f).T)   # [E+H, 4H]
    whhT = np.ascontiguousarray(np.asarray(W_hh, dtype=f).T)   # [H, 4H]
    b4 = (np.asarray(b_ih, dtype=f), np.asarray(b_hh, dtype=f))

    woT = np.zeros((H, VPAD), dtype=f)
    woT[:, :V] = np.asarray(W_out, dtype=f).T
    boP = np.full((VPAD,), -1e30, dtype=f)
    boP[:V] = np.asarray(b_out, dtype=f)

    ident = np.eye(P, dtype=f)
    onesr = np.ones((1, B), dtype=f)

    in_maps = []
    for k in range(NC):
        gcols = np.concatenate(
            [np.arange(g * H + k * HS, g * H + (k + 1) * HS) for g in range(4)])
        in_maps.append({
            "enc": np.ascontiguousarray(enc[k * BS:(k + 1) * BS]),
            "wihT": np.ascontiguousarray(wihT[:, gcols]),
            "whhT": np.ascontiguousarray(whhT[:, gcols]),
            "bihr": np.ascontiguousarray(b4[0][gcols].reshape(1, GS)),
            "bhhr": np.ascontiguousarray(b4[1][gcols].reshape(1, GS)),
            "woT": np.ascontiguousarray(woT[:, k * VS:(k + 1) * VS]),
            "boS": np.ascontiguousarray(boP[k * VS:(k + 1) * VS].reshape(1, VS)),
            "emb": emb,
            "xidx": idx,
            "h0T": h0T,
            "c0s": np.ascontiguousarray(c0[:, k * HS:(k + 1) * HS]),
            "lens": lens,
            "weR": weR,
            "ident": ident,
            "onesr": onesr,
        })
    return in_maps


def kernel(**inputs):
    from concourse.bass_utils import run_bass_kernel_spmd

    nc = _get_program()
    in_maps = _prep_inputs(**inputs)
    res = run_bass_kernel_spmd(nc, in_maps, list(range(NC)))
    outs = res.results

    logprobs = np.concatenate([outs[k]["lp"] for k in range(NC)], axis=1)[:, :V]
    h1 = np.concatenate([outs[k]["h1s"] for k in range(NC)], axis=1)   # [B, H]
    c1 = np.concatenate([outs[k]["c1s"] for k in range(NC)], axis=1)
    aw = np.concatenate([outs[k]["aws"] for k in range(NC)], axis=0)   # [B, S]
    return (logprobs.astype(np.float32), h1[None].astype(np.float32),
            c1[None].astype(np.float32), aw.astype(np.float32))
